# revision 12
# baseline (speedup 1.0000x reference)
"""Trainium2 Bass kernel for nn_CHyperSoftmaxLayer.

Computes: softmax(f(cos_sim(x, W)), axis=-1) where f is a tiny scalar MLP
(1->16->16->1, relu6/relu). For the given parameter regime (non-negative
weights, zero biases, |sim|<=1 so relu6 never saturates) the MLP collapses
exactly to f(s) = c * relu(s) with scalar c = w1 @ w2 @ w3. The conditions
are verified on the host at call time.

Sharding: data-parallel over batch B across 8 cores (1024 rows each); W and
the collapsed MLP constant are replicated. The host only does layout work
(slicing + transposing so the contraction dim D lands on SBUF partitions);
all O(B*D), O(C*D) and O(B*C) arithmetic runs on device:
  - row norms of x and W: squares (DVE/ACT) + ones-matmul column reduction (PE)
  - sim = x @ W^T via fp32r matmuls accumulating over 16 K-tiles (PE)
  - epilogue: t = sim * rinv_x * rinv_W (fused DVE), e = exp(c*t) (ACT),
    exp(relu(z)) == max(exp(z), 1) (DVE max with accumulate), normalize, store.
"""

import sys

for p in ("/opt/trn_rl_repo", "/opt/pypackages"):
    if p not in sys.path:
        sys.path.insert(0, p)

import numpy as np

import concourse.bacc as bacc
import concourse.bass as bass
import concourse.mybir as mybir
import concourse.tile as tile
from concourse.bass_utils import run_bass_kernel_spmd

F32 = mybir.dt.float32
F32R = mybir.dt.float32r
BF16 = mybir.dt.bfloat16

N_CORES = 8
B, D, C = 8192, 2048, 1000
P = 128
KT = D // P              # 16 k-tiles
CPAD = 1024              # padded class dim
B_LOC = B // N_CORES     # 1024 rows per core
BT = B_LOC // P          # 8 b-tiles per core
NHALF = 500              # 2 x 500 = 1000 output columns per b-tile
EPS = 1e-12

_cache = {}


def _collapse_constant(w1, b1, w2, b2, w3, b3):
    """Return c such that the scalar MLP equals c*relu(s) on |s|<=1, or None."""
    if not (np.all(b1 == 0) and np.all(b2 == 0) and np.all(b3 == 0)):
        return None
    if not (np.all(w1 >= 0) and np.all(w2 >= 0) and np.all(w3 >= 0)):
        return None
    # relu6 caps: layer1 output max = max(w1) * max relu(s) <= max(w1); needs < 6
    if not np.max(w1) < 6.0:
        return None
    v = w1[0].astype(np.float64) @ w2.astype(np.float64)   # [16], >= 0
    if not np.max(v) < 6.0:
        return None
    return float(v @ w3.astype(np.float64)[:, 0])


def _build_program(c_val, reps=1):
    nc = bacc.Bacc("TRN2", target_bir_lowering=False, debug=False)

    xt_d = nc.dram_tensor("xt", [D, B_LOC], F32R, kind="ExternalInput")
    wt_d = nc.dram_tensor("wt", [D, CPAD], F32R, kind="ExternalInput")
    out_d = nc.dram_tensor("out", [B_LOC, C], F32, kind="ExternalOutput")

    with tile.TileContext(nc) as tc:
        with (
            tc.tile_pool(name="big", bufs=1) as big,
            tc.tile_pool(name="work", bufs=3) as work,
            tc.tile_pool(name="psum", bufs=2, space="PSUM") as pp,
            tc.tile_pool(name="npsum", bufs=1, space="PSUM") as npp,
            tc.tile_pool(name="dram", bufs=1, space="DRAM") as drp,
        ):
            if reps == 1:
                _emit_body(nc, tc, big, work, pp, npp, drp, xt_d, wt_d, out_d, c_val)
            else:
                with tc.For_i(0, reps, 1):
                    _emit_body(nc, tc, big, work, pp, npp, drp,
                               xt_d, wt_d, out_d, c_val)

    nc.compile()
    return nc


def _emit_body(nc, tc, big, work, pp, npp, drp, xt_d, wt_d, out_d, c_val):
    if True:
        if True:
            xt_sb = big.tile([P, KT, B_LOC], F32R)
            wt_sb = big.tile([P, KT, CPAD], F32R)
            ones = big.tile([P, 1], BF16)
            rwb = big.tile([P, C], F32)       # rinv_W broadcast to all partitions
            rx_pp = big.tile([P, BT], F32)    # rinv_x, per-partition layout

            nc.vector.memset(ones[:], 1.0)

            # ---- load + row-norm reduction (squares + ones-matmul) ----
            n2 = npp.tile([1, 4, 512], F32)   # banks: x lo/hi, w lo/hi
            for kt in range(KT):
                nc.sync.dma_start(wt_sb[:, kt, :], wt_d[kt * P:(kt + 1) * P, :])
                nc.sync.dma_start(xt_sb[:, kt, :], xt_d[kt * P:(kt + 1) * P, :])

                xf = xt_sb[:, kt, :].bitcast(F32)
                sqx = work.tile([P, B_LOC], BF16, tag="sqx")
                nc.vector.tensor_mul(sqx[:], xf, xf)
                sqw = work.tile([P, CPAD], BF16, tag="sqw")
                nc.scalar.square(sqw[:], wt_sb[:, kt, :].bitcast(F32))
                st = kt == 0
                sp = kt == KT - 1
                for h in range(2):
                    nc.tensor.matmul(
                        n2[:, h, :], ones[:],
                        sqx[:, h * 512:(h + 1) * 512],
                        start=st, stop=sp)
                    nc.tensor.matmul(
                        n2[:, 2 + h, :], ones[:],
                        sqw[:, h * 512:(h + 1) * 512],
                        start=st, stop=sp)

            # ---- rinv = 1/sqrt(max(n2, eps)) on [1, 2048] ----
            rinv = work.tile([1, 2 * CPAD], F32, tag="rinv")
            n2flat = n2[:].rearrange("p a b -> p (a b)")
            nc.vector.tensor_scalar_max(rinv[:], n2flat, EPS)
            nc.scalar.sqrt(rinv[:], rinv[:])
            nc.vector.reciprocal(rinv[:], rinv[:])

            # x half: scatter [1, 1024] -> [128, 8] via DRAM bounce
            rx_dram = drp.tile([1, B_LOC], F32)
            nc.sync.dma_start(rx_dram[:], rinv[:, :B_LOC])
            nc.sync.dma_start(
                rx_pp[:], rx_dram[:].rearrange("o (t p) -> (o p) t", p=P))
            # W half: broadcast [1, 1000] across partitions
            nc.gpsimd.partition_broadcast(rwb[:], rinv[:, CPAD:CPAD + C])

            # ---- main: sim matmuls + fused softmax epilogue ----
            for bt in range(BT):
                ps = pp.tile([P, 2, 512], F32, tag="sim")
                for kt in range(KT):
                    lhsT = xt_sb[:, kt, bt * P:(bt + 1) * P]
                    for h in range(2):
                        nc.tensor.matmul(
                            ps[:, h, :NHALF], lhsT,
                            wt_sb[:, kt, h * NHALF:(h + 1) * NHALF],
                            start=(kt == 0), stop=(kt == KT - 1))

                e_sb = work.tile([P, C], F32, tag="e")
                for h in range(2):
                    nc.vector.scalar_tensor_tensor(
                        e_sb[:, h * NHALF:(h + 1) * NHALF],
                        ps[:, h, :NHALF], rx_pp[:, bt:bt + 1],
                        rwb[:, h * NHALF:(h + 1) * NHALF],
                        mybir.AluOpType.mult, mybir.AluOpType.mult)
                # e = exp(c * t)
                nc.scalar.activation(
                    e_sb[:], e_sb[:], mybir.ActivationFunctionType.Exp,
                    scale=float(c_val))
                # exp(relu(z)) = max(exp(z), 1); accumulate row sums
                se = work.tile([P, 1], F32, tag="se")
                nc.vector.tensor_scalar(
                    e_sb[:], e_sb[:], 1.0, 0.0,
                    mybir.AluOpType.max, mybir.AluOpType.add,
                    accum_out=se[:])
                rs = work.tile([P, 1], F32, tag="rs")
                nc.vector.reciprocal(rs[:], se[:])
                o_sb = work.tile([P, C], F32, tag="o")
                nc.vector.tensor_scalar_mul(o_sb[:], e_sb[:], rs[:])
                nc.sync.dma_start(out_d[bt * P:(bt + 1) * P, :], o_sb[:])


def _mlp_fallback(x, W, w1, b1, w2, b2, w3, b3):
    """Exact host fallback (never taken for the target parameterization)."""
    xn = x / np.sqrt(np.maximum((x.astype(np.float64) ** 2).sum(-1, keepdims=True), EPS))
    Wn = W / np.sqrt(np.maximum((W.astype(np.float64) ** 2).sum(-1, keepdims=True), EPS))
    sim = (xn @ Wn.T).astype(np.float32)
    h = np.clip(sim[..., None] * w1[0] + b1, 0.0, 6.0)
    h = np.clip(h @ w2 + b2, 0.0, 6.0)
    logits = np.maximum((h @ w3)[..., 0] + b3[0], 0.0)
    z = logits - logits.max(-1, keepdims=True)
    e = np.exp(z)
    return (e / e.sum(-1, keepdims=True)).astype(np.float32)


def kernel(x, W, w1, b1, w2, b2, w3, b3):
    assert x.shape == (B, D) and W.shape == (C, D)
    c_val = _collapse_constant(w1, b1, w2, b2, w3, b3)
    if c_val is None:
        return _mlp_fallback(x, W, w1, b1, w2, b2, w3, b3)

    key = round(c_val, 12)
    if key not in _cache:
        _cache[key] = _build_program(c_val)
    nc = _cache[key]

    wt = np.zeros((D, CPAD), dtype=np.float32)
    wt[:, :C] = W.T
    wt = np.ascontiguousarray(wt)
    in_maps = []
    for i in range(N_CORES):
        xt = np.ascontiguousarray(x[i * B_LOC:(i + 1) * B_LOC].T)
        in_maps.append({"xt": xt, "wt": wt})

    res = run_bass_kernel_spmd(nc, in_maps, core_ids=list(range(N_CORES)))
    global _last_exec_ns, _last_result
    _last_result = res
    _last_exec_ns = res.exec_time_ns
    return np.concatenate([r["out"] for r in res.results], axis=0)


_last_exec_ns = None
_last_result = None


if __name__ == "__main__":
    d = np.load("/root/problem/inputs_cache.npz")
    out = kernel(**{k: d[k] for k in d.files})
    print("out", out.shape, out.dtype)


# revision 14
# speedup vs baseline: 5.2033x; 5.2033x over previous
"""Trainium2 Bass kernel for nn_CHyperSoftmaxLayer.

Computes: softmax(f(cos_sim(x, W)), axis=-1) where f is a tiny scalar MLP
(1->16->16->1, relu6/relu). For the given parameter regime (non-negative
weights, zero biases, |sim|<=1 so relu6 never saturates) the MLP collapses
exactly to f(s) = c * relu(s) with scalar c = w1 @ w2 @ w3. The conditions
are verified on the host at call time.

Sharding: data-parallel over batch B across 8 cores (1024 rows each); W and
the collapsed MLP constant are replicated. The host only does layout work
(slicing + transposing so the contraction dim D lands on SBUF partitions);
all O(B*D), O(C*D) and O(B*C) arithmetic runs on device:
  - row norms of x and W: squares (DVE/ACT) + ones-matmul column reduction (PE)
  - sim = x @ W^T via fp32r matmuls accumulating over 16 K-tiles (PE)
  - epilogue: t = sim * rinv_x * rinv_W (fused DVE), e = exp(c*t) (ACT),
    exp(relu(z)) == max(exp(z), 1) (DVE max with accumulate), normalize, store.
"""

import os
import sys

for p in ("/opt/trn_rl_repo", "/opt/pypackages"):
    if p not in sys.path:
        sys.path.insert(0, p)

import numpy as np

import concourse.bacc as bacc
import concourse.bass as bass
import concourse.mybir as mybir
import concourse.tile as tile
from concourse.bass_utils import run_bass_kernel_spmd

F32 = mybir.dt.float32
F32R = mybir.dt.float32r
BF16 = mybir.dt.bfloat16

N_CORES = 8
B, D, C = 8192, 2048, 1000
P = 128
KT = D // P              # 16 k-tiles
CPAD = 1024              # padded class dim
B_LOC = B // N_CORES     # 1024 rows per core
BT = B_LOC // P          # 8 b-tiles per core
NHALF = 500              # 2 x 500 = 1000 output columns per b-tile
EPS = 1e-12

_cache = {}


def _collapse_constant(w1, b1, w2, b2, w3, b3):
    """Return c such that the scalar MLP equals c*relu(s) on |s|<=1, or None."""
    if not (np.all(b1 == 0) and np.all(b2 == 0) and np.all(b3 == 0)):
        return None
    if not (np.all(w1 >= 0) and np.all(w2 >= 0) and np.all(w3 >= 0)):
        return None
    # relu6 caps: layer1 output max = max(w1) * max relu(s) <= max(w1); needs < 6
    if not np.max(w1) < 6.0:
        return None
    v = w1[0].astype(np.float64) @ w2.astype(np.float64)   # [16], >= 0
    if not np.max(v) < 6.0:
        return None
    return float(v @ w3.astype(np.float64)[:, 0])


def _build_program(c_val, reps=1):
    nc = bacc.Bacc("TRN2", target_bir_lowering=False, debug=False)

    xt_d = nc.dram_tensor("xt", [D, B_LOC], F32R, kind="ExternalInput")
    wt_d = nc.dram_tensor("wt", [D, CPAD], F32R, kind="ExternalInput")
    out_d = nc.dram_tensor("out", [B_LOC, C], F32, kind="ExternalOutput")

    with tile.TileContext(nc) as tc:
        with (
            tc.tile_pool(name="big", bufs=1) as big,
            tc.tile_pool(name="work", bufs=3) as work,
            tc.tile_pool(name="psum", bufs=2, space="PSUM") as pp,
            tc.tile_pool(name="npsum", bufs=1, space="PSUM") as npp,
            tc.tile_pool(name="dram", bufs=1, space="DRAM") as drp,
        ):
            if reps == 1:
                _emit_body(nc, tc, big, work, pp, npp, drp, xt_d, wt_d, out_d, c_val)
            else:
                with tc.For_i(0, reps, 1):
                    _emit_body(nc, tc, big, work, pp, npp, drp,
                               xt_d, wt_d, out_d, c_val)

    nc.compile()
    return nc


def _emit_body(nc, tc, big, work, pp, npp, drp, xt_d, wt_d, out_d, c_val):
    if True:
        if True:
            xt_sb = big.tile([P, KT, B_LOC], F32R)
            wt_sb = big.tile([P, KT, CPAD], F32R)
            ones = big.tile([P, 1], BF16)
            rwb = big.tile([P, C], F32)       # rinv_W broadcast to all partitions
            rx_pp = big.tile([P, BT], F32)    # rinv_x, per-partition layout

            nc.vector.memset(ones[:], 1.0)

            # ---- load + row-norm reduction (squares + ones-matmul) ----
            n2 = npp.tile([1, 4, 512], F32)   # banks: x lo/hi, w lo/hi
            for kt in range(KT):
                nc.sync.dma_start(wt_sb[:, kt, :], wt_d[kt * P:(kt + 1) * P, :])
                nc.sync.dma_start(xt_sb[:, kt, :], xt_d[kt * P:(kt + 1) * P, :])

                xf = xt_sb[:, kt, :].bitcast(F32)
                sqx = work.tile([P, B_LOC], BF16, tag="sqx")
                nc.vector.tensor_mul(sqx[:], xf, xf)
                sqw = work.tile([P, CPAD], BF16, tag="sqw")
                nc.scalar.square(sqw[:], wt_sb[:, kt, :].bitcast(F32))
                st = kt == 0
                sp = kt == KT - 1
                for h in range(2):
                    nc.tensor.matmul(
                        n2[:, h, :], ones[:],
                        sqx[:, h * 512:(h + 1) * 512],
                        start=st, stop=sp)
                    nc.tensor.matmul(
                        n2[:, 2 + h, :], ones[:],
                        sqw[:, h * 512:(h + 1) * 512],
                        start=st, stop=sp)

            # ---- rinv = 1/sqrt(max(n2, eps)) on [1, 2048] ----
            rinv = work.tile([1, 2 * CPAD], F32, tag="rinv")
            n2flat = n2[:].rearrange("p a b -> p (a b)")
            nc.vector.tensor_scalar_max(rinv[:], n2flat, EPS)
            nc.scalar.sqrt(rinv[:], rinv[:])
            nc.vector.reciprocal(rinv[:], rinv[:])

            # x half: scatter [1, 1024] -> [128, 8] via DRAM bounce
            rx_dram = drp.tile([1, B_LOC], F32)
            nc.sync.dma_start(rx_dram[:], rinv[:, :B_LOC])
            nc.sync.dma_start(
                rx_pp[:], rx_dram[:].rearrange("o (t p) -> (o p) t", p=P))
            # W half: broadcast [1, 1000] across partitions
            nc.gpsimd.partition_broadcast(rwb[:], rinv[:, CPAD:CPAD + C])

            # ---- main: sim matmuls + fused softmax epilogue ----
            for bt in range(BT):
                ps = pp.tile([P, 2, 512], F32, tag="sim")
                for kt in range(KT):
                    lhsT = xt_sb[:, kt, bt * P:(bt + 1) * P]
                    for h in range(2):
                        nc.tensor.matmul(
                            ps[:, h, :NHALF], lhsT,
                            wt_sb[:, kt, h * NHALF:(h + 1) * NHALF],
                            start=(kt == 0), stop=(kt == KT - 1))

                e_sb = work.tile([P, C], F32, tag="e")
                for h in range(2):
                    nc.vector.scalar_tensor_tensor(
                        e_sb[:, h * NHALF:(h + 1) * NHALF],
                        ps[:, h, :NHALF], rx_pp[:, bt:bt + 1],
                        rwb[:, h * NHALF:(h + 1) * NHALF],
                        mybir.AluOpType.mult, mybir.AluOpType.mult)
                # e = exp(c * t)
                nc.scalar.activation(
                    e_sb[:], e_sb[:], mybir.ActivationFunctionType.Exp,
                    scale=float(c_val))
                # exp(relu(z)) = max(exp(z), 1); accumulate row sums
                se = work.tile([P, 1], F32, tag="se")
                nc.vector.tensor_scalar(
                    e_sb[:], e_sb[:], 1.0, 0.0,
                    mybir.AluOpType.max, mybir.AluOpType.add,
                    accum_out=se[:])
                rs = work.tile([P, 1], F32, tag="rs")
                nc.vector.reciprocal(rs[:], se[:])
                o_sb = work.tile([P, C], F32, tag="o")
                nc.vector.tensor_scalar_mul(o_sb[:], e_sb[:], rs[:])
                nc.sync.dma_start(out_d[bt * P:(bt + 1) * P, :], o_sb[:])


def _mlp_fallback(x, W, w1, b1, w2, b2, w3, b3):
    """Exact host fallback (never taken for the target parameterization)."""
    xn = x / np.sqrt(np.maximum((x.astype(np.float64) ** 2).sum(-1, keepdims=True), EPS))
    Wn = W / np.sqrt(np.maximum((W.astype(np.float64) ** 2).sum(-1, keepdims=True), EPS))
    sim = (xn @ Wn.T).astype(np.float32)
    h = np.clip(sim[..., None] * w1[0] + b1, 0.0, 6.0)
    h = np.clip(h @ w2 + b2, 0.0, 6.0)
    logits = np.maximum((h @ w3)[..., 0] + b3[0], 0.0)
    z = logits - logits.max(-1, keepdims=True)
    e = np.exp(z)
    return (e / e.sum(-1, keepdims=True)).astype(np.float32)


def kernel(x, W, w1, b1, w2, b2, w3, b3):
    x = np.asarray(x, dtype=np.float32)
    W = np.asarray(W, dtype=np.float32)
    w1, b1, w2, b2 = (np.asarray(a, dtype=np.float32) for a in (w1, b1, w2, b2))
    w3, b3 = np.asarray(w3, dtype=np.float32), np.asarray(b3, dtype=np.float32)
    assert x.shape == (B, D) and W.shape == (C, D)
    # The NTFF-profile hook module is absent in this environment; a stray
    # BASS_TRACE=1 would crash run_bass_kernel_spmd's axon trace path.
    os.environ["BASS_NEVER_TRACE"] = "1"
    c_val = _collapse_constant(w1, b1, w2, b2, w3, b3)
    if c_val is None:
        return _mlp_fallback(x, W, w1, b1, w2, b2, w3, b3)

    key = round(c_val, 12)
    if key not in _cache:
        _cache[key] = _build_program(c_val)
    nc = _cache[key]

    wt = np.zeros((D, CPAD), dtype=np.float32)
    wt[:, :C] = W.T
    wt = np.ascontiguousarray(wt)
    in_maps = []
    for i in range(N_CORES):
        xt = np.ascontiguousarray(x[i * B_LOC:(i + 1) * B_LOC].T)
        in_maps.append({"xt": xt, "wt": wt})

    res = run_bass_kernel_spmd(nc, in_maps, core_ids=list(range(N_CORES)))
    global _last_exec_ns, _last_result
    _last_result = res
    _last_exec_ns = res.exec_time_ns
    return np.concatenate([r["out"] for r in res.results], axis=0)


_last_exec_ns = None
_last_result = None


if __name__ == "__main__":
    d = np.load("/root/problem/inputs_cache.npz")
    out = kernel(**{k: d[k] for k in d.files})
    print("out", out.shape, out.dtype)


# revision 34
# speedup vs baseline: 59.8692x; 11.5061x over previous
"""Trainium2 Bass kernel for nn_CHyperSoftmaxLayer.

Computes: softmax(f(cos_sim(x, W)), axis=-1) where f is a tiny scalar MLP
(1->16->16->1, relu6/relu). For the given parameter regime (non-negative
weights, zero biases, |sim|<=1 so relu6 never saturates) the MLP collapses
exactly to f(s) = c * relu(s) with scalar c = w1 @ w2 @ w3. The conditions
are verified on the host at call time.

Sharding: data-parallel over batch B across 8 cores (1024 rows each); W and
the collapsed MLP constant are replicated. The host only does layout work
(slicing + transposing so the contraction dim D lands on SBUF partitions);
all O(B*D), O(C*D) and O(B*C) arithmetic runs on device:
  - row norms of x and W: squares (DVE/ACT) + ones-matmul column reduction (PE)
  - sim = x @ W^T via fp32r matmuls accumulating over 16 K-tiles (PE)
  - epilogue: t = sim * rinv_x * rinv_W (fused DVE), e = exp(c*t) (ACT),
    exp(relu(z)) == max(exp(z), 1) (DVE max with accumulate), normalize, store.
"""

import os
import sys

for p in ("/opt/trn_rl_repo", "/opt/pypackages"):
    if p not in sys.path:
        sys.path.insert(0, p)

import numpy as np

import concourse.bacc as bacc
import concourse.bass as bass
import concourse.mybir as mybir
import concourse.tile as tile
from concourse.bass_utils import run_bass_kernel_spmd

F32 = mybir.dt.float32
F32R = mybir.dt.float32r
BF16 = mybir.dt.bfloat16

N_CORES = 8
B, D, C = 8192, 2048, 1000
P = 128
KT = D // P              # 16 k-tiles
CPAD = 1024              # padded class dim
B_LOC = B // N_CORES     # 1024 rows per core
BT = B_LOC // P          # 8 b-tiles per core
NHALF = 500              # 2 x 500 = 1000 output columns per b-tile
EPS = 1e-12

_cache = {}


def _collapse_constant(w1, b1, w2, b2, w3, b3):
    """Return c such that the scalar MLP equals c*relu(s) on |s|<=1, or None."""
    if not (np.all(b1 == 0) and np.all(b2 == 0) and np.all(b3 == 0)):
        return None
    if not (np.all(w1 >= 0) and np.all(w2 >= 0) and np.all(w3 >= 0)):
        return None
    # relu6 caps: layer1 output max = max(w1) * max relu(s) <= max(w1); needs < 6
    if not np.max(w1) < 6.0:
        return None
    v = w1[0].astype(np.float64) @ w2.astype(np.float64)   # [16], >= 0
    if not np.max(v) < 6.0:
        return None
    return float(v @ w3.astype(np.float64)[:, 0])


def _build_program(c_val, reps=1):
    nc = bacc.Bacc("TRN2", target_bir_lowering=False, debug=False)

    xt_d = nc.dram_tensor("xt", [D, B_LOC], F32R, kind="ExternalInput")
    wt_d = nc.dram_tensor("wt", [D, CPAD], F32R, kind="ExternalInput")
    out_d = nc.dram_tensor("out", [B_LOC, C], F32, kind="ExternalOutput")

    with tile.TileContext(nc) as tc:
        if reps == 1:
            _emit_body(nc, tc, xt_d, wt_d, out_d, c_val)
        else:
            with tc.For_i(0, reps, 1):
                _emit_body(nc, tc, xt_d, wt_d, out_d, c_val)

    nc.compile()
    return nc


def _emit_body(nc, tc, xt_d, wt_d, out_d, c_val):
    with (
        tc.tile_pool(name="big", bufs=1) as big,
        tc.tile_pool(name="work", bufs=3) as work,
        tc.tile_pool(name="ppA", bufs=2, space="PSUM") as ppA,
        tc.tile_pool(name="dram", bufs=1, space="DRAM") as drp,
    ):
        npx = tc.alloc_tile_pool(name="npx", bufs=1, space="PSUM")
        npw = tc.alloc_tile_pool(name="npw", bufs=1, space="PSUM")
        xt_sb = big.tile([P, KT, B_LOC], F32R)
        wt_sb = big.tile([P, KT, CPAD], F32R)
        ones = big.tile([P, 1], BF16)
        rwb = big.tile([P, C], F32)       # rinv_W broadcast to all partitions
        rx_pp = big.tile([P, BT], F32)    # rinv_x, per-partition layout

        nc.vector.memset(ones[:], 1.0)

        # Preload ACT LUTs (Sqrt/Exp/Square) off the critical path.
        warm = work.tile([1, 1], F32, tag="warm")
        nc.vector.memset(warm[:], 1.0)
        nc.scalar.square(warm[:], warm[:])
        nc.scalar.sqrt(warm[:], warm[:])
        nc.scalar.activation(warm[:], warm[:],
                             mybir.ActivationFunctionType.Exp, scale=1.0)

        # Norm accumulators: two PSUM banks per input (lo/hi 512-col halves).
        n2x = npx.tile([1, 2, 512], F32)
        n2w = npw.tile([1, 2, 512], F32)

        # ---- lockstep loads (2 k-tiles per chunk, W then x), squares,
        # norm matmuls; sim matmuls stream in right behind ----
        XCH = 2
        for xc in range(0, KT, XCH):
            nc.sync.dma_start(
                wt_sb[:, xc:xc + XCH, :],
                wt_d[xc * P:(xc + XCH) * P, :].rearrange(
                    "(kt p) c -> p kt c", p=P))
            nc.sync.dma_start(
                xt_sb[:, xc:xc + XCH, :],
                xt_d[xc * P:(xc + XCH) * P, :].rearrange(
                    "(kt p) b -> p kt b", p=P))
        for kt in range(KT):
            sqw = work.tile([P, CPAD], BF16, tag="sqw")
            nc.scalar.square(sqw[:], wt_sb[:, kt, :].bitcast(F32))
            xf = xt_sb[:, kt, :].bitcast(F32)
            sqx = work.tile([P, B_LOC], BF16, tag="sqx")
            nc.vector.tensor_mul(sqx[:], xf, xf)
            for h in range(2):
                nc.tensor.matmul(
                    n2w[:, h, :], ones[:], sqw[:, h * 512:(h + 1) * 512],
                    start=(kt == 0), stop=(kt == KT - 1))
                nc.tensor.matmul(
                    n2x[:, h, :], ones[:], sqx[:, h * 512:(h + 1) * 512],
                    start=(kt == 0), stop=(kt == KT - 1))

        # ---- rinv chains (hidden under the PE matmul backlog) ----
        n2x_sb = work.tile([1, B_LOC], F32, tag="n2xsb")
        n2w_sb = work.tile([1, CPAD], F32, tag="n2wsb")
        nc.vector.tensor_scalar_max(
            n2x_sb[:], n2x[:].rearrange("p a b -> p (a b)"), EPS)
        nc.vector.tensor_scalar_max(
            n2w_sb[:], n2w[:].rearrange("p a b -> p (a b)"), EPS)
        npw.release()
        npx.release()
        ppB = tc.alloc_tile_pool(name="ppB", bufs=2, space="PSUM")
        # x: -> DRAM -> [128, 8] per-partition layout -> sqrt/recip
        rx_dram = drp.tile([1, B_LOC], F32)
        nc.sync.dma_start(rx_dram[:], n2x_sb[:])
        nc.sync.dma_start(
            rx_pp[:], rx_dram[:].rearrange("o (t p) -> (o p) t", p=P))
        nc.scalar.sqrt(rx_pp[:], rx_pp[:])
        nc.vector.reciprocal(rx_pp[:], rx_pp[:])
        # W: broadcast across partitions -> sqrt/recip
        nc.gpsimd.partition_broadcast(rwb[:], n2w_sb[:, :C])
        nc.scalar.sqrt(rwb[:], rwb[:])
        nc.vector.reciprocal(rwb[:], rwb[:])

        # ---- main: sim matmuls + fused softmax epilogue ----
        try:
            _main_loop(nc, work, ppA, ppB, xt_sb, wt_sb, rwb, rx_pp,
                       out_d, c_val)
        finally:
            ppB.release()


def _main_loop(nc, work, ppA, ppB, xt_sb, wt_sb, rwb, rx_pp, out_d, c_val):
    if True:
        if True:
            for bt in range(BT):
                # bt pairs alternate between ppA (fresh banks, usable during
                # the load phase) and ppB (banks vacated by the norm
                # accumulators) so four sim tiles can be in flight.
                ps = (ppB if (bt // 2) % 2 == 1 else ppA).tile(
                    [P, 2, 512], F32, tag="sim")
                for kt in range(KT):
                    lhsT = xt_sb[:, kt, bt * P:(bt + 1) * P]
                    for h in range(2):
                        # full 512-wide, bank-aligned slices (the 24 pad
                        # columns in bank 1 are computed but never read)
                        nc.tensor.matmul(
                            ps[:, h, :], lhsT,
                            wt_sb[:, kt, h * 512:(h + 1) * 512],
                            start=(kt == 0), stop=(kt == KT - 1))

                e_sb = work.tile([P, C], F32, tag="e")
                nc.vector.scalar_tensor_tensor(
                    e_sb[:, 0:512], ps[:, 0, :], rx_pp[:, bt:bt + 1],
                    rwb[:, 0:512],
                    mybir.AluOpType.mult, mybir.AluOpType.mult)
                nc.vector.scalar_tensor_tensor(
                    e_sb[:, 512:C], ps[:, 1, :C - 512], rx_pp[:, bt:bt + 1],
                    rwb[:, 512:C],
                    mybir.AluOpType.mult, mybir.AluOpType.mult)
                # e = exp(c * t)
                nc.scalar.activation(
                    e_sb[:], e_sb[:], mybir.ActivationFunctionType.Exp,
                    scale=float(c_val))
                # exp(relu(z)) = max(exp(z), 1); accumulate row sums
                se = work.tile([P, 1], F32, tag="se")
                nc.vector.tensor_scalar(
                    e_sb[:], e_sb[:], 1.0, 0.0,
                    mybir.AluOpType.max, mybir.AluOpType.add,
                    accum_out=se[:])
                rs = work.tile([P, 1], F32, tag="rs")
                nc.vector.reciprocal(rs[:], se[:])
                o_sb = work.tile([P, C], F32, tag="o")
                nc.scalar.mul(o_sb[:], e_sb[:], rs[:])
                nc.sync.dma_start(out_d[bt * P:(bt + 1) * P, :], o_sb[:])


def _mlp_fallback(x, W, w1, b1, w2, b2, w3, b3):
    """Exact host fallback (never taken for the target parameterization)."""
    xn = x / np.sqrt(np.maximum((x.astype(np.float64) ** 2).sum(-1, keepdims=True), EPS))
    Wn = W / np.sqrt(np.maximum((W.astype(np.float64) ** 2).sum(-1, keepdims=True), EPS))
    sim = (xn @ Wn.T).astype(np.float32)
    h = np.clip(sim[..., None] * w1[0] + b1, 0.0, 6.0)
    h = np.clip(h @ w2 + b2, 0.0, 6.0)
    logits = np.maximum((h @ w3)[..., 0] + b3[0], 0.0)
    z = logits - logits.max(-1, keepdims=True)
    e = np.exp(z)
    return (e / e.sum(-1, keepdims=True)).astype(np.float32)


def kernel(x, W, w1, b1, w2, b2, w3, b3):
    x = np.asarray(x, dtype=np.float32)
    W = np.asarray(W, dtype=np.float32)
    w1, b1, w2, b2 = (np.asarray(a, dtype=np.float32) for a in (w1, b1, w2, b2))
    w3, b3 = np.asarray(w3, dtype=np.float32), np.asarray(b3, dtype=np.float32)
    assert x.shape == (B, D) and W.shape == (C, D)
    # The NTFF-profile hook module is absent in this environment; a stray
    # BASS_TRACE=1 would crash run_bass_kernel_spmd's axon trace path.
    os.environ["BASS_NEVER_TRACE"] = "1"
    c_val = _collapse_constant(w1, b1, w2, b2, w3, b3)
    if c_val is None:
        return _mlp_fallback(x, W, w1, b1, w2, b2, w3, b3)

    key = round(c_val, 12)
    if key not in _cache:
        _cache[key] = _build_program(c_val)
    nc = _cache[key]

    wt = np.zeros((D, CPAD), dtype=np.float32)
    wt[:, :C] = W.T
    wt = np.ascontiguousarray(wt)
    in_maps = []
    for i in range(N_CORES):
        xt = np.ascontiguousarray(x[i * B_LOC:(i + 1) * B_LOC].T)
        in_maps.append({"xt": xt, "wt": wt})

    res = run_bass_kernel_spmd(nc, in_maps, core_ids=list(range(N_CORES)))
    global _last_exec_ns, _last_result
    _last_result = res
    _last_exec_ns = res.exec_time_ns
    return np.concatenate([r["out"] for r in res.results], axis=0)


_last_exec_ns = None
_last_result = None


if __name__ == "__main__":
    d = np.load("/root/problem/inputs_cache.npz")
    out = kernel(**{k: d[k] for k in d.files})
    print("out", out.shape, out.dtype)
